# revision 6
# baseline (speedup 1.0000x reference)
"""CenterLoss kernel for Trainium2 (8 NeuronCores, data-parallel over batch).

reference:
    gathered = centers[labels]            # [B, D] gather from [V, D]
    loss = sum((feat - gathered)**2) / B / 2

Sharding: feat/labels split along batch across 8 cores; centers replicated
(each core only touches the rows its labels hit, via indirect DMA gather).
Each core computes per-partition partial sums [128, 1]; the host adds the
8x128 partials and applies the / B / 2 normalization.

Implementation notes (HW-verified):
  - indirect_dma_start is only correct with [P, 1] offset APs (one offset
    per partition, one gathered row per partition); multi-column offset APs
    scramble on hardware. So the 2048-row gather is 16 instructions.
  - Raw engine blocks with manual semaphores (no TileContext): Tile's
    kernel-tail drain + EVSEM butterfly costs ~16us per launch, which would
    dominate this ~20us kernel.
  - Compute instructions may carry at most ONE embedded semaphore wait
    (walrus codegen limit); standalone engine wait_ge instructions are used
    everywhere instead.
"""

import numpy as np

import concourse.bass as bass
import concourse.bacc as bacc
from concourse import mybir

NUM_CLASSES = 100000
D = 256
B = 16384
N_CORES = 8
B_SHARD = B // N_CORES  # 2048
P = 128
T = B_SHARD // P  # 16 sample rows per partition
N_CHUNKS = 4
TC = T // N_CHUNKS  # rows per partition per chunk

_CACHE = {}


def build_nc():
    nc = bacc.Bacc("TRN2", target_bir_lowering=False)
    # feat shard viewed [P, T, D]: partition p holds sample rows
    # [p*T, (p+1)*T); labels[p, t] pairs with feat[p, t, :].
    feat = nc.declare_dram_parameter("feat", [P, T, D], mybir.dt.float32, isOutput=False)
    labels = nc.declare_dram_parameter("labels", [P, T], mybir.dt.int32, isOutput=False)
    centers = nc.declare_dram_parameter(
        "centers", [NUM_CLASSES, D], mybir.dt.float32, isOutput=False
    )
    out = nc.declare_dram_parameter("out", [P, 1], mybir.dt.float32, isOutput=True)

    with (
        nc.semaphore("L") as L,   # labels dma done
        nc.semaphore("F") as F,   # feat dma done
        nc.semaphore("G") as G,   # gather dmas done (16 per gather)
        nc.semaphore("V") as V,   # DVE progress
        nc.semaphore("A") as A,   # ACT progress
        nc.semaphore("O") as O,   # output dma done
        nc.sbuf_tensor("labels_sb", [P, T], mybir.dt.int32) as labels_sb,
        nc.sbuf_tensor("feat_sb", [P, T, D], mybir.dt.float32) as feat_sb,
        nc.sbuf_tensor("gath_sb", [P, T, D], mybir.dt.float32) as gath_sb,
        nc.sbuf_tensor("diff_sb", [P, T, D], mybir.dt.float32) as diff_sb,
        nc.sbuf_tensor("sq_sb", [P, TC, D], mybir.dt.float32) as sq_sb,
        nc.sbuf_tensor("acc_sb", [P, N_CHUNKS], mybir.dt.float32) as acc_sb,
        nc.sbuf_tensor("red_sb", [P, 1], mybir.dt.float32) as red_sb,
    ):
        with nc.Block() as block:

            @block.sync
            def _(sync):
                sync.dma_start(out=labels_sb[:, :], in_=labels[:, :]).then_inc(L, 16)
                sync.dma_start(out=feat_sb[:, :, :], in_=feat[:, :, :]).then_inc(F, 16)
                # tail: wait for DVE to finish the final reduction
                sync.wait_ge(V, N_CHUNKS + 1)
                sync.dma_start(out=out[:, :], in_=red_sb[:, :]).then_inc(O, 16)
                sync.wait_ge(O, 16)

            @block.gpsimd
            def _(gpsimd):
                gpsimd.wait_ge(L, 16)
                for t in range(T):
                    gpsimd.indirect_dma_start(
                        out=gath_sb[:, t, :],
                        out_offset=None,
                        in_=centers[:],
                        in_offset=bass.IndirectOffsetOnAxis(
                            ap=labels_sb[:, t : t + 1], axis=0
                        ),
                    ).then_inc(G, 16)

            @block.vector
            def _(vector):
                vector.wait_ge(F, 16)
                for c in range(N_CHUNKS):
                    lo, hi = c * TC, (c + 1) * TC
                    vector.wait_ge(G, 16 * TC * (c + 1))
                    vector.tensor_sub(
                        out=diff_sb[:, lo:hi, :],
                        in0=feat_sb[:, lo:hi, :],
                        in1=gath_sb[:, lo:hi, :],
                    ).then_inc(V, 1)
                vector.wait_ge(A, N_CHUNKS)
                vector.tensor_reduce(
                    out=red_sb[:, :],
                    in_=acc_sb[:, :],
                    axis=mybir.AxisListType.X,
                    op=mybir.AluOpType.add,
                ).then_inc(V, 1)

            @block.scalar
            def _(scalar):
                for c in range(N_CHUNKS):
                    lo, hi = c * TC, (c + 1) * TC
                    scalar.wait_ge(V, c + 1)
                    scalar.activation(
                        sq_sb[:, :, :],
                        diff_sb[:, lo:hi, :],
                        mybir.ActivationFunctionType.Square,
                        accum_out=acc_sb[:, c : c + 1],
                    ).then_inc(A, 1)

    nc.compile()
    return nc


def _get_nc():
    if "nc" not in _CACHE:
        _CACHE["nc"] = build_nc()
    return _CACHE["nc"]


def make_in_maps(feat, labels, centers):
    feat = np.ascontiguousarray(np.asarray(feat, dtype=np.float32))
    centers = np.ascontiguousarray(np.asarray(centers, dtype=np.float32))
    labels_i32 = np.asarray(labels).astype(np.int32)
    assert feat.shape == (B, D) and labels_i32.shape == (B,)
    assert centers.shape == (NUM_CLASSES, D)
    in_maps = []
    for c in range(N_CORES):
        lo, hi = c * B_SHARD, (c + 1) * B_SHARD
        in_maps.append(
            {
                "feat": feat[lo:hi].reshape(P, T, D),
                "labels": labels_i32[lo:hi].reshape(P, T),
                "centers": centers,
            }
        )
    return in_maps


def kernel(feat, labels, centers):
    from concourse.bass_utils import run_bass_kernel_spmd

    nc = _get_nc()
    in_maps = make_in_maps(feat, labels, centers)
    res = run_bass_kernel_spmd(nc, in_maps, list(range(N_CORES)))
    total = float(sum(np.asarray(r["out"], dtype=np.float64).sum() for r in res.results))
    return np.float32(total / B / 2.0)
